# revision 51
# baseline (speedup 1.0000x reference)
"""Trainium2 Bass kernel for nn_AttnMech (sparse_attention, no-softmax attention).

Math (reference):
  q/k/v = 2x2-stride-2 convs of pose/app_pose/app  -> [B, 4*64, 48, 48]
  attn  = (Q^T K)/8 (no softmax);  out = attn @ V^T
  out   = gamma_h * out; nearest-upsample 2x; concat with pose; 1x1 conv.

Key algebraic restructure (linear attention => reassociate):
  out_h = V_h (Q_h^T K_h)^T / 8 = (V_h K_h^T) Q_h / 8 = G_h Q_h / 8
with G_h = V_h K_h^T a tiny 64x64 Gram matrix.  The per-head projection,
upsample and final 1x1 conv then fold into:
  final = fw1 @ pose_enc + fb + up2x( W_cat @ Q ) ,
  W_cat[:, 64h:64h+64] = (gamma_h/8) * fw2_h @ G_h
so the huge [2304,2304] attention matrices never exist.

Sharding over 8 cores: core c = (image b = c//2, spatial half = c%2).
Partial per-head Gram diagonals are exchanged pairwise with a bf16
AllGather (64 KB payload) and summed inside the W_cat matmul.

Optimizations vs the original AllReduce baseline (95us -> ~70us):
  - K/V convs run "transposed" (image patch as stationary operand, conv
    weights as moving operand) so conv output lands pixel-major in PSUM
    and feeds the Gram matmuls directly -- no PE transposes, no
    PSUM->SBUF transpose copies.  K/V biases are added by [P,C]
    broadcast tiles shipped in wpack; strips are host-permuted
    block-major so each 128-pixel chunk is one contiguous stationary
    slice and arrives in its own DMA.
  - AllReduce -> AllGather of the bf16 Gram diagonals packed to 32KB;
    rank contributions are summed inside partition-offset W_cat
    matmuls.  The collective firmware has a fixed ~11us trigger-to-mesh
    latency plus a ~43-48us boot floor, so the gram stage store is
    prioritized over the xq input tail (which is semaphore-gated behind
    it) to fire the trigger as early as possible.
  - Output is stored bf16 (host casts back to fp32): halves store bytes.
  - up2x column duplication is folded into the z matmul via a stride-0
    moving AP; row duplication via a stride-0 broadcast read in the
    final adds (innermost 96 contiguous, so DVE runs near full rate).
    z spills go mostly to the scalar engine; each oc's adds are issued
    right after its z tiles so the vector FIFO never parks an add
    behind later spills, and the last add/store is split for a short
    trailing chain.
  - PE is pre-warmed with junk matmuls sized to the initial DMA wait so
    the HAM clock gate is released before the first conv; the scalar
    activation table is preloaded the same way.
"""

import os
import sys

for _p in ("/opt/trn_rl_repo", "/root/.axon_site/_ro/trn_rl_repo"):
    if os.path.isdir(_p) and _p not in sys.path:
        sys.path.insert(0, _p)

import numpy as np

import concourse.mybir as mybir
import concourse.tile as tile
from concourse import bacc, bass2jax

F32 = mybir.dt.float32
F16 = mybir.dt.float16
BF16 = mybir.dt.bfloat16
ADD = mybir.AluOpType.add
BYPASS = mybir.AluOpType.bypass
IDENT = mybir.ActivationFunctionType.Identity

P = 128          # partitions
C = 256          # channels
W_IMG = 96       # full-res width
RH = 48          # rows per half (full-res)
FR = RH * W_IMG  # 4608 flat half-image
NI = 24          # local downsampled rows
NJ = 48          # downsampled cols
NLOC = NI * NJ   # 1152 local attn pixels
NT = 3           # strips of 16 full-res rows
TW = 384
SLEN = 2 * 16 * W_IMG  # strip len per partition (both ic chunks) 3072
NCH = 9          # 128-pixel chunks of the local grid (3 strips x 3 col blocks)
NZT = 6          # z tiles per out-channel chunk (4 ds-rows, col-dupped, each)

# wpack layout (per partition, bf16 words) + separate fp32 bias blob
KW_O = 0
KBB_O = 2048
VW_O = 2304
VBB_O = 4352
QW_O = 4608
FW1_O = 6656
FW2_O = 7168
WLEN = 7680
QB_O, FB_O = 0, 2
WSLEN = 4

_CACHED_NC = None
_RUNNER = None


def _make_runner(nc, n_cores=8):
    """Like bass2jax.run_bass_via_pjrt, but inputs are pre-placed on the
    devices (parallel transfer + aligned core start) and the jitted
    executable is cached across calls."""
    import jax
    from jax.experimental.shard_map import shard_map
    from jax.sharding import Mesh, NamedSharding, PartitionSpec

    bass2jax.install_neuronx_cc_hook()

    partition_name = (
        nc.partition_id_tensor.name if nc.partition_id_tensor else None
    )
    in_names, out_names, out_avals = [], [], []
    for alloc in nc.m.functions[0].allocations:
        if not isinstance(alloc, mybir.MemoryLocationSet):
            continue
        name = alloc.memorylocations[0].name
        if alloc.kind == "ExternalInput":
            if name != partition_name:
                in_names.append(name)
        elif alloc.kind == "ExternalOutput":
            out_avals.append(
                jax.core.ShapedArray(
                    tuple(alloc.tensor_shape), mybir.dt.np(alloc.dtype)
                )
            )
            out_names.append(name)
    n_params = len(in_names)
    all_in = tuple(in_names + out_names)
    if partition_name is not None:
        all_in = all_in + (partition_name,)

    def _body(*args):
        operands = list(args)
        if partition_name is not None:
            operands.append(bass2jax.partition_id_tensor())
        return tuple(
            bass2jax._bass_exec_p.bind(
                *operands,
                out_avals=tuple(out_avals),
                in_names=all_in,
                out_names=tuple(out_names),
                lowering_input_output_aliases=(),
                sim_require_finite=True,
                sim_require_nnan=True,
                nc=nc,
            )
        )

    devices = jax.devices()[:n_cores]
    mesh = Mesh(np.asarray(devices), ("core",))
    nspec = n_params + len(out_names)
    donate = tuple(range(n_params, nspec))
    sharded = jax.jit(
        shard_map(
            _body,
            mesh=mesh,
            in_specs=(PartitionSpec("core"),) * nspec,
            out_specs=(PartitionSpec("core"),) * len(out_names),
            check_rep=False,
        ),
        donate_argnums=donate,
        keep_unused=True,
    )
    sh = NamedSharding(mesh, PartitionSpec("core"))

    def run(in_maps):
        concat_in = [
            jax.device_put(
                np.concatenate([np.asarray(m[nm]) for m in in_maps], axis=0), sh
            )
            for nm in in_names
        ]
        import jax.numpy as jnp

        concat_zeros = [
            jax.device_put(
                jnp.zeros((n_cores * a.shape[0], *a.shape[1:]), a.dtype), sh
            )
            for a in out_avals
        ]
        jax.block_until_ready(concat_in)
        jax.block_until_ready(concat_zeros)
        try:
            out_arrs = sharded(*concat_in, *concat_zeros)
            jax.block_until_ready(out_arrs)
        except Exception:
            # transient runtime desync (seen on the first launch after a
            # prior process used collectives) — one retry recovers
            concat_zeros = [
                jax.device_put(
                    jnp.zeros((n_cores * a.shape[0], *a.shape[1:]), a.dtype), sh
                )
                for a in out_avals
            ]
            jax.block_until_ready(concat_zeros)
            out_arrs = sharded(*concat_in, *concat_zeros)
            jax.block_until_ready(out_arrs)
        return [
            {
                nm: np.asarray(out_arrs[i]).reshape(n_cores, *out_avals[i].shape)[c]
                for i, nm in enumerate(out_names)
            }
            for c in range(n_cores)
        ]

    return run


def _build():
    nc = bacc.Bacc("TRN2", target_bir_lowering=False, debug=False, num_devices=8)

    xq_d = nc.dram_tensor("xq", [P, 2, FR], BF16, kind="ExternalInput").ap()
    xk_d = nc.dram_tensor("xk", [P, NT, SLEN], BF16, kind="ExternalInput").ap()
    xv_d = nc.dram_tensor("xv", [P, NT, SLEN], BF16, kind="ExternalInput").ap()
    wpack_d = nc.dram_tensor("wpack", [P, WLEN], BF16, kind="ExternalInput").ap()
    wps_d = nc.dram_tensor("wps", [P, WSLEN], F32, kind="ExternalInput").ap()

    out_d = nc.dram_tensor("out", [P, 2, FR], BF16, kind="ExternalOutput").ap()

    gpart_d = nc.dram_tensor("g_part", [P, P], BF16).ap()
    gag_d = nc.dram_tensor("g_ag", [2 * P, P], BF16).ap()

    from concourse.tile_rust import add_dep_helper

    RG = [[0, 1], [2, 3], [4, 5], [6, 7]]

    with tile.TileContext(nc) as tc:
        with (
            tc.tile_pool(name="const", bufs=1) as cpool,
            tc.tile_pool(name="img", bufs=1) as ipool,
            tc.tile_pool(name="mid", bufs=2) as mpool,
            tc.tile_pool(name="work", bufs=1) as wpool,
            tc.tile_pool(name="outp", bufs=2) as opool,
            tc.tile_pool(name="ps", bufs=2, space="PSUM") as psp,
        ):
            wp = cpool.tile([P, WLEN], BF16, tag="wp")
            wps_sb = cpool.tile([P, WSLEN], F32, tag="wps")
            actw = cpool.tile([P, 2], F32, tag="actw")

            # warm the scalar-engine activation table during the first DMAs
            # so the first real activation doesn't eat the 1.3us table load
            nc.gpsimd.memzero(actw[:])
            nc.scalar.activation(actw[:], actw[:], IDENT, bias=0.0, scale=1.0)

            # PE pre-warm sized to the initial DMA wait: junk matmuls on a
            # zeroed tile release the HAM clock gate so the first conv runs
            # at 2.4 GHz; they finish before the xk data lands.
            zj = cpool.tile([P, 512], BF16, tag="zj")
            nc.gpsimd.memzero(zj[:])
            # 24 iterations bridge the whole DMA wait (~6.6->13.4us) so the
            # HAM never re-throttles between prewarm end and the first conv
            # even on runs where the input DMA lands late
            for i in range(24):
                psj = psp.tile([P, 512], F32, tag="cv", name=f"junk{i}")
                nc.tensor.matmul(
                    psj[:], zj[:, :P], zj[:], start=True, stop=True,
                    skip_group_check=True,
                )

            # ---- phase-A loads: K-conv prerequisites only, then xk ----
            nc.sync.dma_start(wps_sb[:], wps_d)
            nc.sync.dma_start(wp[:, KW_O:VW_O], wpack_d[:, KW_O:VW_O])
            xk_sb = ipool.tile([P, NT, SLEN], BF16, tag="xk")
            BL = SLEN // 3  # one block-chunk (icc*tap*128 pix) per DMA
            xk_dmas = [
                nc.sync.dma_start(
                    xk_sb[:, s, b * BL : (b + 1) * BL],
                    xk_d[:, s, b * BL : (b + 1) * BL],
                )
                for s in range(NT)
                for b in range(3)
            ]

            qw_v = wp[:, QW_O : QW_O + 2048].rearrange(
                "p (i d o) -> p i d o", i=2, d=4
            )
            kw_v = wp[:, KW_O : KW_O + 2048].rearrange(
                "p (i d o) -> p i d o", i=2, d=4
            )
            vw_v = wp[:, VW_O : VW_O + 2048].rearrange(
                "p (i d o) -> p i d o", i=2, d=4
            )
            fw1_v = wp[:, FW1_O : FW1_O + 512].rearrange("p (i o) -> p i o", i=2)
            fw2_v = wp[:, FW2_O : FW2_O + 512].rearrange("p (i o) -> p i o", i=2)
            kbb_v = wp[:, KBB_O : KBB_O + C]
            vbb_v = wp[:, VBB_O : VBB_O + C]

            def sca(off):  # [P, 1] fp32 per-partition scalar view
                return wps_sb[:, off : off + 2]

            def conv_chunk_mms(src_sb, w_v, s, b, nm):
                """Transposed 2x2/s2 conv for one 128-pixel chunk: image
                patch stationary, weights moving; output [pix, oc] in a
                fresh PSUM tile (returned; bias-add is the caller's).
                Host pre-permutes each strip to [block, icc, tap, 128pix]
                so the stationary is a contiguous [P, 128] slice and each
                block's data arrives in its own DMA."""
                ps = psp.tile([P, C], F32, tag="cv", name=f"c{nm}{s}{b}")
                sv = src_sb[:, s].rearrange(
                    "p (b i d x) -> p b i d x", b=3, i=2, d=4
                )
                first_mm = None
                for icc in range(2):
                    for dd in range(4):
                        mm = nc.tensor.matmul(
                            ps[:],
                            sv[:, b, icc, dd, :],
                            w_v[:, icc, dd, :],
                            start=(icc == 0 and dd == 0),
                            stop=(icc == 1 and dd == 3),
                        )
                        if first_mm is None:
                            first_mm = mm
                return ps, first_mm

            # ---- K conv: 9 chunks -> kt_all [pix, chunk, ch] ----
            kt_all = wpool.tile([P, NCH, C], BF16, tag="ktall")
            for s in range(NT):
                for b in range(3):
                    t = 3 * s + b
                    ps, _ = conv_chunk_mms(xk_sb, kw_v, s, b, "k")
                    nc.vector.tensor_tensor(kt_all[:, t, :], ps[:], kbb_v, ADD)

            # ---- phase-B loads (serialized behind the K strips) ----
            xv_sb = ipool.tile([P, NT, SLEN], BF16, tag="xv")
            d = nc.sync.dma_start(wp[:, VW_O:QW_O], wpack_d[:, VW_O:QW_O])
            add_dep_helper(d.ins, xk_dmas[-1].ins, sync=False, reason="phase loads")
            xv_dmas = []
            for s in range(NT):
                for b in range(3):
                    d = nc.sync.dma_start(
                        xv_sb[:, s, b * BL : (b + 1) * BL],
                        xv_d[:, s, b * BL : (b + 1) * BL],
                    )
                    add_dep_helper(
                        d.ins, xk_dmas[-1].ins, sync=False, reason="phase loads"
                    )
                    xv_dmas.append(d)

            # ---- V conv + streamed Gram accumulation, software-pipelined
            #      by one chunk so the in-order PE never waits on the
            #      vector bias-add (gram MMs for chunk t issue after the
            #      conv MMs of chunk t+1) ----
            gps = [
                psp.tile([P, P], F32, tag="gram", name=f"gps{g}")
                for g in range(2)
            ]
            gmm = None
            pend = None  # (t, vt) whose gram MMs are not yet issued

            def issue_gram(t, vt):
                nonlocal gmm
                for g in range(2):
                    gmm = nc.tensor.matmul(
                        gps[g][:],
                        vt[:, g * P : (g + 1) * P],
                        kt_all[:, t, g * P : (g + 1) * P],
                        start=(t == 0),
                        stop=(t == NCH - 1),
                        skip_group_check=True,
                    )

            for s in range(NT):
                for b in range(3):
                    t = 3 * s + b
                    ps, _ = conv_chunk_mms(xv_sb, vw_v, s, b, "v")
                    if pend is not None:
                        issue_gram(*pend)
                    vt = mpool.tile([P, C], BF16, tag="vt", name=f"vt{t}")
                    nc.vector.tensor_tensor(vt[:], ps[:], vbb_v, ADD)
                    pend = (t, vt)
            issue_gram(*pend)

            # ---- Gram exchange: stage per-head diagonal blocks packed to
            #      [P, 2g, 64] bf16 (32 KB), AllGather pairwise, sum ranks
            #      inside partition-offset W_cat matmuls.  Copies run on
            #      the scalar engine (idle here; vector still busy with
            #      the V-phase bias adds). ----
            gstage = wpool.tile([P, 2, 64], BF16, tag="gstage")
            for g in range(2):
                for hh in range(2):
                    r0 = 64 * hh
                    nc.scalar.copy(
                        gstage[r0 : r0 + 64, g, :],
                        gps[g][r0 : r0 + 64, r0 : r0 + 64],
                    )
            gpart_dma = nc.scalar.dma_start(gpart_d, gstage[:])
            nc.gpsimd.collective_compute(
                "AllGather", BYPASS, replica_groups=RG,
                ins=[gpart_d], outs=[gag_d],
            )
            g_sb = wpool.tile([P, 2, P], BF16, tag="gsb")
            nc.sync.dma_start(
                g_sb[:], gag_d.rearrange("(r p) c -> p r c", r=2)
            )

            # ---- phase-C loads (Q conv + pose prerequisites) ----
            # only the xq tail is held behind the gram stage store; the
            # rest streams right behind xv so Q conv is never starved
            d = nc.sync.dma_start(wp[:, QW_O:WLEN], wpack_d[:, QW_O:WLEN])
            add_dep_helper(d.ins, xv_dmas[-1].ins, sync=False, reason="phase loads")
            xq_sb = ipool.tile([P, 2, FR], BF16, tag="xq")
            for s in range(NT):
                for icc in range(2):
                    d = nc.sync.dma_start(
                        xq_sb[:, icc, s * 1536 : (s + 1) * 1536],
                        xq_d[:, icc, s * 1536 : (s + 1) * 1536],
                    )
                    if s == NT - 1:
                        add_dep_helper(
                            d.ins, gpart_dma.ins, reason="gram store priority"
                        )
                    else:
                        add_dep_helper(
                            d.ins, xv_dmas[-1].ins, sync=False,
                            reason="phase loads",
                        )

            # ---- Q conv (fills the collective latency window) ----
            q_sb = wpool.tile([P, 2, NLOC], BF16, tag="q")
            xqv = [
                xq_sb[:, icc, :].rearrange("p (r w) -> p r w", w=W_IMG)
                for icc in range(2)
            ]
            # strip-major order: both channel chunks of strip 0/1 run
            # before any strip-2 tile, giving the DMA-gated xq tail
            # (held behind the gram stage store) time to land
            for nt in range(NT):
                for qcc in range(2):
                    ps = psp.tile(
                        [P, TW], F32, tag="qp", bufs=4, name=f"qq{qcc}{nt}"
                    )
                    psv = ps[:].rearrange("p (i j) -> p i j", j=NJ)
                    first = True
                    for icc in range(2):
                        for dd in range(4):
                            di, dj = dd // 2, dd % 2
                            mm = nc.tensor.matmul(
                                psv,
                                qw_v[:, icc, dd, qcc * P : (qcc + 1) * P],
                                xqv[icc][:, 16 * nt + di : 16 * nt + 16 : 2, dj::2],
                                start=first,
                                stop=(icc == 1 and dd == 3),
                            )
                            if first and qcc == 0 and nt == 0:
                                add_dep_helper(
                                    mm.ins, gmm.ins, sync=False,
                                    reason="pin Q conv after Gram",
                                )
                            first = False
                    if nt % 2:
                        nc.scalar.activation(
                            q_sb[:, qcc, nt * TW : (nt + 1) * TW], ps[:],
                            IDENT, bias=sca(QB_O)[:, qcc : qcc + 1], scale=1.0,
                        )
                    else:
                        nc.vector.tensor_tensor(
                            q_sb[:, qcc, nt * TW : (nt + 1) * TW], ps[:],
                            sca(QB_O)[:, qcc : qcc + 1].to_broadcast([P, TW]),
                            ADD,
                        )

            # ---- pose term: stage = fw1 @ pose + fb (bf16, fb folded) ----
            stage = wpool.tile([P, 2, FR], BF16, tag="stage")
            pmm = None
            PW = 512
            # groups of 3 tiles so each fw1 chunk stays loaded for 3
            # consecutive matmuls
            for oc in range(2):
                for g3 in range(3):
                    pss = [
                        psp.tile(
                            [P, PW], F32, tag="qp", bufs=4, name=f"pp{oc}{g3}{t}"
                        )
                        for t in range(3)
                    ]
                    for icc in range(2):
                        for t in range(3):
                            ot = 3 * g3 + t
                            pmm = nc.tensor.matmul(
                                pss[t][:],
                                fw1_v[:, icc, oc * P : (oc + 1) * P],
                                xq_sb[:, icc, ot * PW : (ot + 1) * PW],
                                start=(icc == 0),
                                stop=(icc == 1),
                                skip_group_check=True,
                            )
                    for t in range(3):
                        ot = 3 * g3 + t
                        dsl = stage[:, oc, ot * PW : (ot + 1) * PW]
                        if t % 2:
                            nc.scalar.activation(
                                dsl, pss[t][:], IDENT,
                                bias=sca(FB_O)[:, oc : oc + 1], scale=1.0,
                            )
                        else:
                            nc.vector.tensor_tensor(
                                dsl, pss[t][:],
                                sca(FB_O)[:, oc : oc + 1].to_broadcast([P, PW]),
                                ADD,
                            )

            # ---- W_cat^T = sum_r blockdiag(G_r) @ fw2'^T (gamma/8 folded) ----
            w_sb = wpool.tile([P, 2, C], BF16, tag="w")
            for g in range(2):
                psw = psp.tile([P, C], F32, tag="qp", bufs=4, name=f"psw{g}")
                for r in range(2):
                    for hh in range(2):
                        r0 = 64 * hh
                        mm = nc.tensor.matmul(
                            psw[r0 : r0 + 64, :],
                            g_sb[r0 : r0 + 64, r, g * 64 : (g + 1) * 64],
                            fw2_v[r0 : r0 + 64, g, :],
                            start=(r == 0),
                            stop=(r == 1),
                            skip_group_check=True,
                        )
                        if g == 0 and r == 0 and hh == 0:
                            add_dep_helper(
                                mm.ins, pmm.ins, sync=False,
                                reason="pin W_cat after pose",
                            )
                if g:
                    nc.scalar.copy(w_sb[:, g, :], psw[:])
                else:
                    nc.vector.tensor_copy(w_sb[:, g, :], psw[:])

            # ---- z = W_cat^T.T @ Q, column-dup folded into the matmul ----
            qv = [
                q_sb[:, g, :].rearrange("p (i j) -> p i j", j=NJ)
                for g in range(2)
            ]
            zt = [
                wpool.tile([P, NI * W_IMG], BF16, tag=f"zt{oc}", name=f"zt{oc}")
                for oc in range(2)
            ]
            # ---- z matmuls + spills, with each oc's two final adds issued
            #      right after its tiles so the vector FIFO never parks an
            #      add behind later spills.  out = stage + row-dup(zt). ----
            for oc in range(2):
                for t6 in range(NZT):
                    # alternate psum tags: 2+4 ring slots keep the z matmul
                    # stream ahead of the spills; spills go mostly to the
                    # scalar engine so the vector stays free for the adds
                    tg, nb = (("cv", 2), ("qp", 4))[t6 % 2]
                    psz = psp.tile(
                        [P, TW], F32, tag=tg, bufs=nb, name=f"pz{oc}{t6}"
                    )
                    for g in range(2):
                        rhs = qv[g][:, 4 * t6 : 4 * t6 + 4, :, None].to_broadcast(
                            [P, 4, NJ, 2]
                        )
                        nc.tensor.matmul(
                            psz[:],
                            w_sb[:, g, oc * P : (oc + 1) * P],
                            rhs,
                            start=(g == 0),
                            stop=(g == 1),
                        )
                    dsl = zt[oc][:, t6 * TW : (t6 + 1) * TW]
                    if t6 in (0, 1):
                        # vector handles the two EARLIEST tiles so its FIFO
                        # reaches the adds without waiting on a late spill
                        nc.vector.tensor_copy(dsl, psz[:])
                    else:
                        nc.scalar.copy(dsl, psz[:])
                obuf = opool.tile([P, FR], BF16, tag="obuf", name=f"ob{oc}")
                # last half-oc is split in two so the trailing add+store
                # chain after the final z spill is as short as possible
                parts = ((0, 2), (2, 4)) if oc == 0 else ((0, 2), (2, 3), (3, 4))
                for p0, p1 in parts:
                    sl = slice(p0 * FR // 4, p1 * FR // 4)
                    ni = 6 * (p1 - p0)
                    ov = obuf[:, sl].rearrange(
                        "p (i ri x) -> p i ri x", ri=2, x=W_IMG
                    )
                    stv = stage[:, oc, sl].rearrange(
                        "p (i ri x) -> p i ri x", ri=2, x=W_IMG
                    )
                    zv = zt[oc][:, p0 * FR // 8 : p1 * FR // 8].rearrange(
                        "p (i x) -> p i x", x=W_IMG
                    )[:, :, None, :].to_broadcast([P, ni, 2, W_IMG])
                    nc.vector.tensor_tensor(ov, stv, zv, ADD)
                    nc.sync.dma_start(out_d[:, oc, sl], obuf[:, sl])

    nc.compile()
    return nc


def _prep_inputs(inputs):
    """Build the 8 per-core input maps (host-side shard + weight packing)."""
    import ml_dtypes

    f = np.float32
    b16 = ml_dtypes.bfloat16
    qw, qb = np.asarray(inputs["qw"], f), np.asarray(inputs["qb"], f)
    kw, kb = np.asarray(inputs["kw"], f), np.asarray(inputs["kb"], f)
    vw, vb = np.asarray(inputs["vw"], f), np.asarray(inputs["vb"], f)
    gamma = np.asarray(inputs["gamma"], f)
    fw, fb = np.asarray(inputs["fw"], f), np.asarray(inputs["fb"], f)
    pose = np.asarray(inputs["pose_enc"], f)
    app_pose = np.asarray(inputs["app_pose_enc"], f)
    app = np.asarray(inputs["app_enc"], f)

    wpack = np.zeros((P, WLEN), dtype=b16)
    wps = np.zeros((P, WSLEN), dtype=f)

    def packw(dst_off, w):
        # w [oc, ic, 2, 2] -> [p, icc, dd, oc]
        t = w.transpose(1, 2, 3, 0).reshape(2, P, 4, C).transpose(1, 0, 2, 3)
        wpack[:, dst_off : dst_off + 2048] = t.reshape(P, 2048).astype(b16)

    packw(QW_O, qw)
    packw(KW_O, kw)
    packw(VW_O, vw)
    wpack[:, KBB_O : KBB_O + C] = np.broadcast_to(kb, (P, C)).astype(b16)
    wpack[:, VBB_O : VBB_O + C] = np.broadcast_to(vb, (P, C)).astype(b16)
    wpack[:, FW1_O : FW1_O + 512] = (
        fw[:, :C, 0, 0].T.reshape(2, P, C).transpose(1, 0, 2).reshape(P, 512)
    ).astype(b16)
    gsc = (np.repeat(gamma.astype(np.float64), 64) / 8.0)[:, None]
    fw2s = (fw[:, C:, 0, 0].T.astype(np.float64) * gsc).astype(f)
    wpack[:, FW2_O : FW2_O + 512] = (
        fw2s.reshape(2, P, C).transpose(1, 0, 2).reshape(P, 512)
    ).astype(b16)
    wps[:, QB_O : QB_O + 2] = qb.reshape(2, P).T
    wps[:, FB_O : FB_O + 2] = fb.reshape(2, P).T

    def shard_q(x, b, h):  # [p, icc, fr]
        halfimg = x[b, :, RH * h : RH * (h + 1), :].reshape(2, P, FR)
        return halfimg.transpose(1, 0, 2).astype(b16)

    def shard_kv(x, b, h):  # [p, strip, (block, icc, tap, 128pix)]
        # permute so each (strip, block, icc, tap) chunk's 128 pixels are
        # contiguous: stationary operand of the transposed conv; block
        # outermost so each 16-ds-col block ships as its own DMA.
        hi = x[b, :, RH * h : RH * (h + 1), :]
        h8 = hi.reshape(2, P, NT, 8, 2, 3, 16, 2)  # icc p s dr di bl dc dj
        return (
            h8.transpose(1, 2, 5, 0, 4, 7, 3, 6)   # p s bl icc di dj dr dc
            .reshape(P, NT, SLEN)
            .astype(b16)
        )

    in_maps = []
    for c in range(8):
        b, h = c // 2, c % 2
        in_maps.append({
            "xq": shard_q(pose, b, h),
            "xk": shard_kv(app_pose, b, h),
            "xv": shard_kv(app, b, h),
            "wpack": wpack,
            "wps": wps,
        })
    return in_maps


def _get_runner():
    global _CACHED_NC, _RUNNER
    if _CACHED_NC is None:
        _CACHED_NC = _build()
    if _RUNNER is None:
        _RUNNER = _make_runner(_CACHED_NC)
    return _RUNNER


def _assemble(results):
    out = np.empty((4, C, W_IMG, W_IMG), dtype=np.float32)
    for c in range(8):
        b, h = c // 2, c % 2
        o = results[c]["out"]  # [P, 2, FR] bf16
        out[b, :, RH * h : RH * (h + 1), :] = (
            o.astype(np.float32).transpose(1, 0, 2).reshape(C, RH, W_IMG)
        )
    return out


def kernel(**inputs):
    run = _get_runner()
    in_maps = _prep_inputs(inputs)
    return _assemble(run(in_maps))


# revision 52
# speedup vs baseline: 1.0223x; 1.0223x over previous
"""Trainium2 Bass kernel for nn_AttnMech (sparse_attention, no-softmax attention).

Math (reference):
  q/k/v = 2x2-stride-2 convs of pose/app_pose/app  -> [B, 4*64, 48, 48]
  attn  = (Q^T K)/8 (no softmax);  out = attn @ V^T
  out   = gamma_h * out; nearest-upsample 2x; concat with pose; 1x1 conv.

Key algebraic restructure (linear attention => reassociate):
  out_h = V_h (Q_h^T K_h)^T / 8 = (V_h K_h^T) Q_h / 8 = G_h Q_h / 8
with G_h = V_h K_h^T a tiny 64x64 Gram matrix.  The per-head projection,
upsample and final 1x1 conv then fold into:
  final = fw1 @ pose_enc + fb + up2x( W_cat @ Q ) ,
  W_cat[:, 64h:64h+64] = (gamma_h/8) * fw2_h @ G_h
so the huge [2304,2304] attention matrices never exist.

Sharding over 8 cores: core c = (image b = c//2, spatial half = c%2).
Partial per-head Gram diagonals are exchanged pairwise with a bf16
AllGather (64 KB payload) and summed inside the W_cat matmul.

Optimizations vs the original AllReduce baseline (95us -> ~70us):
  - K/V convs run "transposed" (image patch as stationary operand, conv
    weights as moving operand) so conv output lands pixel-major in PSUM
    and feeds the Gram matmuls directly -- no PE transposes, no
    PSUM->SBUF transpose copies.  K/V biases are added by [P,C]
    broadcast tiles shipped in wpack; strips are host-permuted
    block-major so each 128-pixel chunk is one contiguous stationary
    slice and arrives in its own DMA.
  - AllReduce -> AllGather of the bf16 Gram diagonals packed to 32KB;
    rank contributions are summed inside partition-offset W_cat
    matmuls.  The collective firmware has a fixed ~11us trigger-to-mesh
    latency plus a ~43-48us boot floor, so the gram stage store is
    prioritized over the xq input tail (which is semaphore-gated behind
    it) to fire the trigger as early as possible.
  - Output is stored bf16 (host casts back to fp32): halves store bytes.
  - up2x column duplication is folded into the z matmul via a stride-0
    moving AP; row duplication via a stride-0 broadcast read in the
    final adds (innermost 96 contiguous, so DVE runs near full rate).
    z spills go mostly to the scalar engine; each oc's adds are issued
    right after its z tiles so the vector FIFO never parks an add
    behind later spills, and the last add/store is split for a short
    trailing chain.
  - PE is pre-warmed with junk matmuls sized to the initial DMA wait so
    the HAM clock gate is released before the first conv; the scalar
    activation table is preloaded the same way.
"""

import os
import sys

for _p in ("/opt/trn_rl_repo", "/root/.axon_site/_ro/trn_rl_repo"):
    if os.path.isdir(_p) and _p not in sys.path:
        sys.path.insert(0, _p)

import numpy as np

import concourse.mybir as mybir
import concourse.tile as tile
from concourse import bacc, bass2jax

F32 = mybir.dt.float32
F16 = mybir.dt.float16
BF16 = mybir.dt.bfloat16
ADD = mybir.AluOpType.add
BYPASS = mybir.AluOpType.bypass
IDENT = mybir.ActivationFunctionType.Identity

P = 128          # partitions
C = 256          # channels
W_IMG = 96       # full-res width
RH = 48          # rows per half (full-res)
FR = RH * W_IMG  # 4608 flat half-image
NI = 24          # local downsampled rows
NJ = 48          # downsampled cols
NLOC = NI * NJ   # 1152 local attn pixels
NT = 3           # strips of 16 full-res rows
TW = 384
SLEN = 2 * 16 * W_IMG  # strip len per partition (both ic chunks) 3072
NCH = 9          # 128-pixel chunks of the local grid (3 strips x 3 col blocks)
NZT = 6          # z tiles per out-channel chunk (4 ds-rows, col-dupped, each)

# wpack layout (per partition, bf16 words) + separate fp32 bias blob
KW_O = 0
KBB_O = 2048
VW_O = 2304
VBB_O = 4352
QW_O = 4608
FW1_O = 6656
FW2_O = 7168
WLEN = 7680
QB_O, FB_O = 0, 2
WSLEN = 4

_CACHED_NC = None
_RUNNER = None


def _make_runner(nc, n_cores=8):
    """Like bass2jax.run_bass_via_pjrt, but inputs are pre-placed on the
    devices (parallel transfer + aligned core start) and the jitted
    executable is cached across calls."""
    import jax
    from jax.experimental.shard_map import shard_map
    from jax.sharding import Mesh, NamedSharding, PartitionSpec

    bass2jax.install_neuronx_cc_hook()

    partition_name = (
        nc.partition_id_tensor.name if nc.partition_id_tensor else None
    )
    in_names, out_names, out_avals = [], [], []
    for alloc in nc.m.functions[0].allocations:
        if not isinstance(alloc, mybir.MemoryLocationSet):
            continue
        name = alloc.memorylocations[0].name
        if alloc.kind == "ExternalInput":
            if name != partition_name:
                in_names.append(name)
        elif alloc.kind == "ExternalOutput":
            out_avals.append(
                jax.core.ShapedArray(
                    tuple(alloc.tensor_shape), mybir.dt.np(alloc.dtype)
                )
            )
            out_names.append(name)
    n_params = len(in_names)
    all_in = tuple(in_names + out_names)
    if partition_name is not None:
        all_in = all_in + (partition_name,)

    def _body(*args):
        operands = list(args)
        if partition_name is not None:
            operands.append(bass2jax.partition_id_tensor())
        return tuple(
            bass2jax._bass_exec_p.bind(
                *operands,
                out_avals=tuple(out_avals),
                in_names=all_in,
                out_names=tuple(out_names),
                lowering_input_output_aliases=(),
                sim_require_finite=True,
                sim_require_nnan=True,
                nc=nc,
            )
        )

    devices = jax.devices()[:n_cores]
    mesh = Mesh(np.asarray(devices), ("core",))
    nspec = n_params + len(out_names)
    donate = tuple(range(n_params, nspec))
    sharded = jax.jit(
        shard_map(
            _body,
            mesh=mesh,
            in_specs=(PartitionSpec("core"),) * nspec,
            out_specs=(PartitionSpec("core"),) * len(out_names),
            check_rep=False,
        ),
        donate_argnums=donate,
        keep_unused=True,
    )
    sh = NamedSharding(mesh, PartitionSpec("core"))

    def run(in_maps):
        concat_in = [
            jax.device_put(
                np.concatenate([np.asarray(m[nm]) for m in in_maps], axis=0), sh
            )
            for nm in in_names
        ]
        import jax.numpy as jnp

        concat_zeros = [
            jax.device_put(
                jnp.zeros((n_cores * a.shape[0], *a.shape[1:]), a.dtype), sh
            )
            for a in out_avals
        ]
        jax.block_until_ready(concat_in)
        jax.block_until_ready(concat_zeros)
        try:
            out_arrs = sharded(*concat_in, *concat_zeros)
            jax.block_until_ready(out_arrs)
        except Exception:
            # transient runtime desync (seen on the first launch after a
            # prior process used collectives) — one retry recovers
            concat_zeros = [
                jax.device_put(
                    jnp.zeros((n_cores * a.shape[0], *a.shape[1:]), a.dtype), sh
                )
                for a in out_avals
            ]
            jax.block_until_ready(concat_zeros)
            out_arrs = sharded(*concat_in, *concat_zeros)
            jax.block_until_ready(out_arrs)
        return [
            {
                nm: np.asarray(out_arrs[i]).reshape(n_cores, *out_avals[i].shape)[c]
                for i, nm in enumerate(out_names)
            }
            for c in range(n_cores)
        ]

    return run


def _build():
    nc = bacc.Bacc("TRN2", target_bir_lowering=False, debug=False, num_devices=8)

    xq_d = nc.dram_tensor("xq", [P, 2, FR], BF16, kind="ExternalInput").ap()
    xk_d = nc.dram_tensor("xk", [P, NT, SLEN], BF16, kind="ExternalInput").ap()
    xv_d = nc.dram_tensor("xv", [P, NT, SLEN], BF16, kind="ExternalInput").ap()
    wpack_d = nc.dram_tensor("wpack", [P, WLEN], BF16, kind="ExternalInput").ap()
    wps_d = nc.dram_tensor("wps", [P, WSLEN], F32, kind="ExternalInput").ap()

    out_d = nc.dram_tensor("out", [P, 2, FR], BF16, kind="ExternalOutput").ap()

    gpart_d = nc.dram_tensor("g_part", [P, P], BF16).ap()
    gag_d = nc.dram_tensor("g_ag", [2 * P, P], BF16).ap()

    from concourse.tile_rust import add_dep_helper

    RG = [[0, 1], [2, 3], [4, 5], [6, 7]]

    with tile.TileContext(nc) as tc:
        with (
            tc.tile_pool(name="const", bufs=1) as cpool,
            tc.tile_pool(name="img", bufs=1) as ipool,
            tc.tile_pool(name="mid", bufs=2) as mpool,
            tc.tile_pool(name="work", bufs=1) as wpool,
            tc.tile_pool(name="outp", bufs=2) as opool,
            tc.tile_pool(name="ps", bufs=2, space="PSUM") as psp,
        ):
            wp = cpool.tile([P, WLEN], BF16, tag="wp")
            wps_sb = cpool.tile([P, WSLEN], F32, tag="wps")
            actw = cpool.tile([P, 2], F32, tag="actw")

            # warm the scalar-engine activation table during the first DMAs
            # so the first real activation doesn't eat the 1.3us table load
            nc.gpsimd.memzero(actw[:])
            nc.scalar.activation(actw[:], actw[:], IDENT, bias=0.0, scale=1.0)

            # PE pre-warm sized to the initial DMA wait: junk matmuls on a
            # zeroed tile release the HAM clock gate so the first conv runs
            # at 2.4 GHz; they finish before the xk data lands.
            zj = cpool.tile([P, 512], BF16, tag="zj")
            nc.gpsimd.memzero(zj[:])
            # 18 iterations bridge the whole DMA wait (~6.6->14us) so the
            # HAM never re-throttles between prewarm end and the first conv
            # even on runs where the input DMA lands late
            for i in range(18):
                psj = psp.tile([P, 512], F32, tag="cv", name=f"junk{i}")
                nc.tensor.matmul(
                    psj[:], zj[:, :P], zj[:], start=True, stop=True,
                    skip_group_check=True,
                )

            # ---- phase-A loads: K-conv prerequisites only, then xk ----
            nc.sync.dma_start(wps_sb[:], wps_d)
            nc.sync.dma_start(wp[:, KW_O:VW_O], wpack_d[:, KW_O:VW_O])
            xk_sb = ipool.tile([P, NT, SLEN], BF16, tag="xk")
            BL = SLEN // 3  # one block-chunk (icc*tap*128 pix) per DMA
            xk_dmas = [
                nc.sync.dma_start(
                    xk_sb[:, s, b * BL : (b + 1) * BL],
                    xk_d[:, s, b * BL : (b + 1) * BL],
                )
                for s in range(NT)
                for b in range(3)
            ]

            qw_v = wp[:, QW_O : QW_O + 2048].rearrange(
                "p (i d o) -> p i d o", i=2, d=4
            )
            kw_v = wp[:, KW_O : KW_O + 2048].rearrange(
                "p (i d o) -> p i d o", i=2, d=4
            )
            vw_v = wp[:, VW_O : VW_O + 2048].rearrange(
                "p (i d o) -> p i d o", i=2, d=4
            )
            fw1_v = wp[:, FW1_O : FW1_O + 512].rearrange("p (i o) -> p i o", i=2)
            fw2_v = wp[:, FW2_O : FW2_O + 512].rearrange("p (i o) -> p i o", i=2)
            kbb_v = wp[:, KBB_O : KBB_O + C]
            vbb_v = wp[:, VBB_O : VBB_O + C]

            def sca(off):  # [P, 1] fp32 per-partition scalar view
                return wps_sb[:, off : off + 2]

            def conv_chunk_mms(src_sb, w_v, s, b, nm):
                """Transposed 2x2/s2 conv for one 128-pixel chunk: image
                patch stationary, weights moving; output [pix, oc] in a
                fresh PSUM tile (returned; bias-add is the caller's).
                Host pre-permutes each strip to [block, icc, tap, 128pix]
                so the stationary is a contiguous [P, 128] slice and each
                block's data arrives in its own DMA."""
                ps = psp.tile([P, C], F32, tag="cv", name=f"c{nm}{s}{b}")
                sv = src_sb[:, s].rearrange(
                    "p (b i d x) -> p b i d x", b=3, i=2, d=4
                )
                first_mm = None
                for icc in range(2):
                    for dd in range(4):
                        mm = nc.tensor.matmul(
                            ps[:],
                            sv[:, b, icc, dd, :],
                            w_v[:, icc, dd, :],
                            start=(icc == 0 and dd == 0),
                            stop=(icc == 1 and dd == 3),
                        )
                        if first_mm is None:
                            first_mm = mm
                return ps, first_mm

            # ---- K conv: 9 chunks -> kt_all [pix, chunk, ch] ----
            kt_all = wpool.tile([P, NCH, C], BF16, tag="ktall")
            for s in range(NT):
                for b in range(3):
                    t = 3 * s + b
                    ps, _ = conv_chunk_mms(xk_sb, kw_v, s, b, "k")
                    nc.vector.tensor_tensor(kt_all[:, t, :], ps[:], kbb_v, ADD)

            # ---- phase-B loads (serialized behind the K strips) ----
            xv_sb = ipool.tile([P, NT, SLEN], BF16, tag="xv")
            d = nc.sync.dma_start(wp[:, VW_O:QW_O], wpack_d[:, VW_O:QW_O])
            add_dep_helper(d.ins, xk_dmas[-1].ins, sync=False, reason="phase loads")
            xv_dmas = []
            for s in range(NT):
                for b in range(3):
                    d = nc.sync.dma_start(
                        xv_sb[:, s, b * BL : (b + 1) * BL],
                        xv_d[:, s, b * BL : (b + 1) * BL],
                    )
                    add_dep_helper(
                        d.ins, xk_dmas[-1].ins, sync=False, reason="phase loads"
                    )
                    xv_dmas.append(d)

            # ---- V conv + streamed Gram accumulation, software-pipelined
            #      by one chunk so the in-order PE never waits on the
            #      vector bias-add (gram MMs for chunk t issue after the
            #      conv MMs of chunk t+1) ----
            gps = [
                psp.tile([P, P], F32, tag="gram", name=f"gps{g}")
                for g in range(2)
            ]
            gmm = None
            pend = None  # (t, vt) whose gram MMs are not yet issued

            def issue_gram(t, vt):
                nonlocal gmm
                for g in range(2):
                    gmm = nc.tensor.matmul(
                        gps[g][:],
                        vt[:, g * P : (g + 1) * P],
                        kt_all[:, t, g * P : (g + 1) * P],
                        start=(t == 0),
                        stop=(t == NCH - 1),
                        skip_group_check=True,
                    )

            for s in range(NT):
                for b in range(3):
                    t = 3 * s + b
                    ps, _ = conv_chunk_mms(xv_sb, vw_v, s, b, "v")
                    if pend is not None:
                        issue_gram(*pend)
                    vt = mpool.tile([P, C], BF16, tag="vt", name=f"vt{t}")
                    nc.vector.tensor_tensor(vt[:], ps[:], vbb_v, ADD)
                    pend = (t, vt)
            issue_gram(*pend)

            # ---- Gram exchange: stage per-head diagonal blocks packed to
            #      [P, 2g, 64] bf16 (32 KB), AllGather pairwise, sum ranks
            #      inside partition-offset W_cat matmuls.  Copies run on
            #      the scalar engine (idle here; vector still busy with
            #      the V-phase bias adds). ----
            gstage = wpool.tile([P, 2, 64], BF16, tag="gstage")
            for g in range(2):
                for hh in range(2):
                    r0 = 64 * hh
                    nc.scalar.copy(
                        gstage[r0 : r0 + 64, g, :],
                        gps[g][r0 : r0 + 64, r0 : r0 + 64],
                    )
            gpart_dma = nc.scalar.dma_start(gpart_d, gstage[:])
            nc.gpsimd.collective_compute(
                "AllGather", BYPASS, replica_groups=RG,
                ins=[gpart_d], outs=[gag_d],
            )
            g_sb = wpool.tile([P, 2, P], BF16, tag="gsb")
            nc.sync.dma_start(
                g_sb[:], gag_d.rearrange("(r p) c -> p r c", r=2)
            )

            # ---- phase-C loads (Q conv + pose prerequisites) ----
            # only the xq tail is held behind the gram stage store; the
            # rest streams right behind xv so Q conv is never starved
            d = nc.sync.dma_start(wp[:, QW_O:WLEN], wpack_d[:, QW_O:WLEN])
            add_dep_helper(d.ins, xv_dmas[-1].ins, sync=False, reason="phase loads")
            xq_sb = ipool.tile([P, 2, FR], BF16, tag="xq")
            for s in range(NT):
                for icc in range(2):
                    d = nc.sync.dma_start(
                        xq_sb[:, icc, s * 1536 : (s + 1) * 1536],
                        xq_d[:, icc, s * 1536 : (s + 1) * 1536],
                    )
                    if s == NT - 1:
                        add_dep_helper(
                            d.ins, gpart_dma.ins, reason="gram store priority"
                        )
                    else:
                        add_dep_helper(
                            d.ins, xv_dmas[-1].ins, sync=False,
                            reason="phase loads",
                        )

            # ---- Q conv (fills the collective latency window) ----
            q_sb = wpool.tile([P, 2, NLOC], BF16, tag="q")
            xqv = [
                xq_sb[:, icc, :].rearrange("p (r w) -> p r w", w=W_IMG)
                for icc in range(2)
            ]
            # strip-major order: both channel chunks of strip 0/1 run
            # before any strip-2 tile, giving the DMA-gated xq tail
            # (held behind the gram stage store) time to land
            for nt in range(NT):
                for qcc in range(2):
                    ps = psp.tile(
                        [P, TW], F32, tag="qp", bufs=4, name=f"qq{qcc}{nt}"
                    )
                    psv = ps[:].rearrange("p (i j) -> p i j", j=NJ)
                    first = True
                    for icc in range(2):
                        for dd in range(4):
                            di, dj = dd // 2, dd % 2
                            mm = nc.tensor.matmul(
                                psv,
                                qw_v[:, icc, dd, qcc * P : (qcc + 1) * P],
                                xqv[icc][:, 16 * nt + di : 16 * nt + 16 : 2, dj::2],
                                start=first,
                                stop=(icc == 1 and dd == 3),
                            )
                            if first and qcc == 0 and nt == 0:
                                add_dep_helper(
                                    mm.ins, gmm.ins, sync=False,
                                    reason="pin Q conv after Gram",
                                )
                            first = False
                    if nt % 2:
                        nc.scalar.activation(
                            q_sb[:, qcc, nt * TW : (nt + 1) * TW], ps[:],
                            IDENT, bias=sca(QB_O)[:, qcc : qcc + 1], scale=1.0,
                        )
                    else:
                        nc.vector.tensor_tensor(
                            q_sb[:, qcc, nt * TW : (nt + 1) * TW], ps[:],
                            sca(QB_O)[:, qcc : qcc + 1].to_broadcast([P, TW]),
                            ADD,
                        )

            # ---- pose term: stage = fw1 @ pose + fb (bf16, fb folded) ----
            stage = wpool.tile([P, 2, FR], BF16, tag="stage")
            pmm = None
            PW = 512
            # groups of 3 tiles so each fw1 chunk stays loaded for 3
            # consecutive matmuls
            for oc in range(2):
                for g3 in range(3):
                    pss = [
                        psp.tile(
                            [P, PW], F32, tag="qp", bufs=4, name=f"pp{oc}{g3}{t}"
                        )
                        for t in range(3)
                    ]
                    for icc in range(2):
                        for t in range(3):
                            ot = 3 * g3 + t
                            pmm = nc.tensor.matmul(
                                pss[t][:],
                                fw1_v[:, icc, oc * P : (oc + 1) * P],
                                xq_sb[:, icc, ot * PW : (ot + 1) * PW],
                                start=(icc == 0),
                                stop=(icc == 1),
                                skip_group_check=True,
                            )
                    for t in range(3):
                        ot = 3 * g3 + t
                        dsl = stage[:, oc, ot * PW : (ot + 1) * PW]
                        if t % 2:
                            nc.scalar.activation(
                                dsl, pss[t][:], IDENT,
                                bias=sca(FB_O)[:, oc : oc + 1], scale=1.0,
                            )
                        else:
                            nc.vector.tensor_tensor(
                                dsl, pss[t][:],
                                sca(FB_O)[:, oc : oc + 1].to_broadcast([P, PW]),
                                ADD,
                            )

            # ---- W_cat^T = sum_r blockdiag(G_r) @ fw2'^T (gamma/8 folded) ----
            w_sb = wpool.tile([P, 2, C], BF16, tag="w")
            for g in range(2):
                psw = psp.tile([P, C], F32, tag="qp", bufs=4, name=f"psw{g}")
                for r in range(2):
                    for hh in range(2):
                        r0 = 64 * hh
                        mm = nc.tensor.matmul(
                            psw[r0 : r0 + 64, :],
                            g_sb[r0 : r0 + 64, r, g * 64 : (g + 1) * 64],
                            fw2_v[r0 : r0 + 64, g, :],
                            start=(r == 0),
                            stop=(r == 1),
                            skip_group_check=True,
                        )
                        if g == 0 and r == 0 and hh == 0:
                            add_dep_helper(
                                mm.ins, pmm.ins, sync=False,
                                reason="pin W_cat after pose",
                            )
                if g:
                    nc.scalar.copy(w_sb[:, g, :], psw[:])
                else:
                    nc.vector.tensor_copy(w_sb[:, g, :], psw[:])

            # ---- z = W_cat^T.T @ Q, column-dup folded into the matmul ----
            qv = [
                q_sb[:, g, :].rearrange("p (i j) -> p i j", j=NJ)
                for g in range(2)
            ]
            zt = [
                wpool.tile([P, NI * W_IMG], BF16, tag=f"zt{oc}", name=f"zt{oc}")
                for oc in range(2)
            ]
            # ---- z matmuls + spills, with each oc's two final adds issued
            #      right after its tiles so the vector FIFO never parks an
            #      add behind later spills.  out = stage + row-dup(zt). ----
            for oc in range(2):
                for t6 in range(NZT):
                    # alternate psum tags: 2+4 ring slots keep the z matmul
                    # stream ahead of the spills; spills go mostly to the
                    # scalar engine so the vector stays free for the adds
                    tg, nb = (("cv", 2), ("qp", 4))[t6 % 2]
                    psz = psp.tile(
                        [P, TW], F32, tag=tg, bufs=nb, name=f"pz{oc}{t6}"
                    )
                    for g in range(2):
                        rhs = qv[g][:, 4 * t6 : 4 * t6 + 4, :, None].to_broadcast(
                            [P, 4, NJ, 2]
                        )
                        nc.tensor.matmul(
                            psz[:],
                            w_sb[:, g, oc * P : (oc + 1) * P],
                            rhs,
                            start=(g == 0),
                            stop=(g == 1),
                        )
                    dsl = zt[oc][:, t6 * TW : (t6 + 1) * TW]
                    if t6 in (0, 1):
                        # vector handles the two EARLIEST tiles so its FIFO
                        # reaches the adds without waiting on a late spill
                        nc.vector.tensor_copy(dsl, psz[:])
                    else:
                        nc.scalar.copy(dsl, psz[:])
                obuf = opool.tile([P, FR], BF16, tag="obuf", name=f"ob{oc}")
                # last half-oc is split in two so the trailing add+store
                # chain after the final z spill is as short as possible
                parts = ((0, 2), (2, 4)) if oc == 0 else ((0, 2), (2, 3), (3, 4))
                for p0, p1 in parts:
                    sl = slice(p0 * FR // 4, p1 * FR // 4)
                    ni = 6 * (p1 - p0)
                    ov = obuf[:, sl].rearrange(
                        "p (i ri x) -> p i ri x", ri=2, x=W_IMG
                    )
                    stv = stage[:, oc, sl].rearrange(
                        "p (i ri x) -> p i ri x", ri=2, x=W_IMG
                    )
                    zv = zt[oc][:, p0 * FR // 8 : p1 * FR // 8].rearrange(
                        "p (i x) -> p i x", x=W_IMG
                    )[:, :, None, :].to_broadcast([P, ni, 2, W_IMG])
                    nc.vector.tensor_tensor(ov, stv, zv, ADD)
                    nc.sync.dma_start(out_d[:, oc, sl], obuf[:, sl])

    nc.compile()
    return nc


def _prep_inputs(inputs):
    """Build the 8 per-core input maps (host-side shard + weight packing)."""
    import ml_dtypes

    f = np.float32
    b16 = ml_dtypes.bfloat16
    qw, qb = np.asarray(inputs["qw"], f), np.asarray(inputs["qb"], f)
    kw, kb = np.asarray(inputs["kw"], f), np.asarray(inputs["kb"], f)
    vw, vb = np.asarray(inputs["vw"], f), np.asarray(inputs["vb"], f)
    gamma = np.asarray(inputs["gamma"], f)
    fw, fb = np.asarray(inputs["fw"], f), np.asarray(inputs["fb"], f)
    pose = np.asarray(inputs["pose_enc"], f)
    app_pose = np.asarray(inputs["app_pose_enc"], f)
    app = np.asarray(inputs["app_enc"], f)

    wpack = np.zeros((P, WLEN), dtype=b16)
    wps = np.zeros((P, WSLEN), dtype=f)

    def packw(dst_off, w):
        # w [oc, ic, 2, 2] -> [p, icc, dd, oc]
        t = w.transpose(1, 2, 3, 0).reshape(2, P, 4, C).transpose(1, 0, 2, 3)
        wpack[:, dst_off : dst_off + 2048] = t.reshape(P, 2048).astype(b16)

    packw(QW_O, qw)
    packw(KW_O, kw)
    packw(VW_O, vw)
    wpack[:, KBB_O : KBB_O + C] = np.broadcast_to(kb, (P, C)).astype(b16)
    wpack[:, VBB_O : VBB_O + C] = np.broadcast_to(vb, (P, C)).astype(b16)
    wpack[:, FW1_O : FW1_O + 512] = (
        fw[:, :C, 0, 0].T.reshape(2, P, C).transpose(1, 0, 2).reshape(P, 512)
    ).astype(b16)
    gsc = (np.repeat(gamma.astype(np.float64), 64) / 8.0)[:, None]
    fw2s = (fw[:, C:, 0, 0].T.astype(np.float64) * gsc).astype(f)
    wpack[:, FW2_O : FW2_O + 512] = (
        fw2s.reshape(2, P, C).transpose(1, 0, 2).reshape(P, 512)
    ).astype(b16)
    wps[:, QB_O : QB_O + 2] = qb.reshape(2, P).T
    wps[:, FB_O : FB_O + 2] = fb.reshape(2, P).T

    def shard_q(x, b, h):  # [p, icc, fr]
        halfimg = x[b, :, RH * h : RH * (h + 1), :].reshape(2, P, FR)
        return halfimg.transpose(1, 0, 2).astype(b16)

    def shard_kv(x, b, h):  # [p, strip, (block, icc, tap, 128pix)]
        # permute so each (strip, block, icc, tap) chunk's 128 pixels are
        # contiguous: stationary operand of the transposed conv; block
        # outermost so each 16-ds-col block ships as its own DMA.
        hi = x[b, :, RH * h : RH * (h + 1), :]
        h8 = hi.reshape(2, P, NT, 8, 2, 3, 16, 2)  # icc p s dr di bl dc dj
        return (
            h8.transpose(1, 2, 5, 0, 4, 7, 3, 6)   # p s bl icc di dj dr dc
            .reshape(P, NT, SLEN)
            .astype(b16)
        )

    in_maps = []
    for c in range(8):
        b, h = c // 2, c % 2
        in_maps.append({
            "xq": shard_q(pose, b, h),
            "xk": shard_kv(app_pose, b, h),
            "xv": shard_kv(app, b, h),
            "wpack": wpack,
            "wps": wps,
        })
    return in_maps


def _get_runner():
    global _CACHED_NC, _RUNNER
    if _CACHED_NC is None:
        _CACHED_NC = _build()
    if _RUNNER is None:
        _RUNNER = _make_runner(_CACHED_NC)
    return _RUNNER


def _assemble(results):
    out = np.empty((4, C, W_IMG, W_IMG), dtype=np.float32)
    for c in range(8):
        b, h = c // 2, c % 2
        o = results[c]["out"]  # [P, 2, FR] bf16
        out[b, :, RH * h : RH * (h + 1), :] = (
            o.astype(np.float32).transpose(1, 0, 2).reshape(C, RH, W_IMG)
        )
    return out


def kernel(**inputs):
    run = _get_runner()
    in_maps = _prep_inputs(inputs)
    return _assemble(run(in_maps))
